# revision 8
# baseline (speedup 1.0000x reference)
"""Trainium2 Bass kernel for nn_DecoderStack (B=4,T=1024,D=1024,H=16,DK=DV=64,FF=4096).

Sharding over 8 NeuronCores: core c -> batch b=c//2, half h=c%2.
Each core computes, for its batch:
  - MHA1 (q=k=v=y) restricted to its 8 heads, through its Wo row-slice (+y/2)
  - MHA2 (q=y, k=v=x) same head split
  - FFN restricted to its 2048 FF rows (+b_out/2)
All three [T,D] partial sums are written to a DRAM bounce buffer laid out
[2(owner t-half), 3(branch), T/2, D]; a ReduceScatter over core pairs
{2b,2b+1} sums them and hands each core its t-half.  The cheap sub_norm chain
(out1/out2/out) then runs on each core's T/2 rows, which are returned and
reassembled host-side.

Matmuls run in bf16 (full PE rate) with fp32 PSUM accumulation; softmax is
over the QUERY axis (faithful to the reference), which maps to the free axis
of the scoresT[s,t] layout, so exp+denominator fuse into one ACT pass.

SBUF is tight (~188KB/partition usable); non-overlapping-lifetime tensors
share pool tags (xT<->wot, wqt<->h1), W_in streams in per-f-chunk, and the
FFN runs one T-half at a time so h1 only ever holds half the intermediate.
"""

import sys

for _p in ("/opt/trn_rl_repo", "/root/.axon_site"):
    if _p not in sys.path:
        sys.path.insert(0, _p)

import contextlib

import numpy as np

import concourse.bass as bass
import concourse.bacc as bacc
import concourse.tile as tile
from concourse import mybir
from concourse.bass_utils import run_bass_kernel_spmd

# ---------------------------------------------------------------- config

B, T, D, H, DK, DV, FF = 4, 1024, 1024, 16, 64, 64, 4096
P = 128
FP32 = mybir.dt.float32
BF16 = mybir.dt.bfloat16
NPBF16 = mybir.dt.np(BF16)


class Cfg:
    def __init__(self, T_=T, D_=D, FF_=FF):
        self.T = T_
        self.D = D_
        self.FF = FF_
        self.NT = T_ // P          # t-tiles (also s-tiles)
        self.ND = D_ // P          # d-chunks
        self.HT = T_ // 2 // P     # t-tiles per half
        self.FFH = FF_ // 2        # FF rows per core
        self.NF = self.FFH // P    # f-chunks per core
        self.HK = 8 * DK           # qk width for 8 heads
        self.HV = 8 * DV           # v width for 8 heads
        self.TH = T_ // 2          # matmul free-dim half


def build_program(cfg: Cfg, n_cores: int = 8, compile: bool = True):
    """Build (and optionally compile) the single SPMD program all cores run."""
    nc = bacc.Bacc("TRN2", target_bir_lowering=False, debug=False,
                   num_devices=n_cores)
    NT, ND, NF, HT, TH = cfg.NT, cfg.ND, cfg.NF, cfg.HT, cfg.TH
    Tq, DN = cfg.T, cfg.D
    NTH = 2                   # number of free-dim halves of T
    NDH = DN // TH            # d output chunks of width TH per row-tile

    def dram_in(name, shape, dt=BF16):
        return nc.dram_tensor(name, shape, dt, kind="ExternalInput")

    yT = dram_in("yT", [P, ND, Tq])           # yT[p,dc,t] = y[t, dc*P+p]
    xT = dram_in("xT", [P, ND, Tq])
    yh = dram_in("yh", [P, NT, DN], FP32)     # y/2 in natural-layout tiles
    wq1 = dram_in("wq1", [P, ND, cfg.HK])     # head-pair lhsT, wq pre-scaled 1/8
    wk1 = dram_in("wk1", [P, ND, cfg.HK])
    wv1 = dram_in("wv1", [P, ND, cfg.HV])
    wo1 = dram_in("wo1", [P, 4, DN])          # wo[p,i,d] = Wo[512*h + i*P + p, d]
    wq2 = dram_in("wq2", [P, ND, cfg.HK])
    wk2 = dram_in("wk2", [P, ND, cfg.HK])
    wv2 = dram_in("wv2", [P, ND, cfg.HV])
    wo2 = dram_in("wo2", [P, 4, DN])
    wi = dram_in("wi", [P, ND, cfg.FFH])      # W_in rows slice, lhsT[d, f]
    wot = dram_in("wot", [P, NF, DN])         # W_out.T rows slice, moving [f, d]
    bi = dram_in("bi", [P, NF], FP32)         # b_in slice, partition layout
    bo = dram_in("bo", [1, DN], FP32)         # b_out / 2
    out = nc.dram_tensor("out", [TH, DN], FP32, kind="ExternalOutput")

    with tile.TileContext(nc) as tc:
        with contextlib.ExitStack() as ctx:
            # static, one buffer per tag
            p1 = ctx.enter_context(tc.tile_pool(name="p1", bufs=1))
            # xT (16KB) then wot (32KB) share one 32KB slot
            xw = ctx.enter_context(tc.tile_pool(name="xw", bufs=1))
            # wqt (8KB) then h1 halves (16KB) share one 16KB slot
            qh = ctx.enter_context(tc.tile_pool(name="qh", bufs=1))
            expp = ctx.enter_context(tc.tile_pool(name="expp", bufs=2))
            wic = ctx.enter_context(tc.tile_pool(name="wic", bufs=2))
            rows = ctx.enter_context(tc.tile_pool(name="rows", bufs=5))
            small = ctx.enter_context(tc.tile_pool(name="small", bufs=2))
            psum = ctx.enter_context(tc.tile_pool(name="psum", bufs=4, space="PSUM"))
            psc = ctx.enter_context(tc.tile_pool(name="psc", bufs=2, space="PSUM"))
            dram = ctx.enter_context(tc.tile_pool(name="dram", bufs=1, space="DRAM"))

            bounce_in = dram.tile([2 * 3 * TH, DN], FP32)
            bounce_out = dram.tile([3 * TH, DN], FP32)

            # persistent activations / constants
            yT_sb = p1.tile([P, ND, Tq], BF16, tag="yT")
            nc.sync.dma_start(yT_sb[:], yT[:])
            xT_sb = xw.tile([P, ND, Tq], BF16, tag="xw")
            nc.sync.dma_start(xT_sb[:], xT[:])
            bo_sb = p1.tile([P, DN], FP32, tag="bo")
            bo_ap = bo[:]
            nc.sync.dma_start(
                bo_sb[:],
                bass.AP(tensor=bo_ap.tensor, offset=bo_ap.offset,
                        ap=[[0, P]] + list(bo_ap.ap[1:])),
            )
            bi_sb = p1.tile([P, NF], FP32, tag="bi")
            nc.sync.dma_start(bi_sb[:], bi[:])

            def mha(wq_d, wk_d, wv_d, wo_d, kvT_sb, branch, add_yh):
                # ---- weights for this attention (single-buffered tags; the
                # second attention's DMA starts as soon as the slot frees)
                wq_sb = p1.tile([P, ND, cfg.HK], BF16, tag="wq")
                nc.sync.dma_start(wq_sb[:], wq_d[:])
                wk_sb = p1.tile([P, ND, cfg.HK], BF16, tag="wk")
                nc.sync.dma_start(wk_sb[:], wk_d[:])
                wv_sb = p1.tile([P, ND, cfg.HV], BF16, tag="wv")
                nc.sync.dma_start(wv_sb[:], wv_d[:])
                wo_sb = p1.tile([P, 4, DN], BF16, tag="wo")
                nc.sync.dma_start(wo_sb[:], wo_d[:])

                # ---- Q/K projections per head-pair: out [P(2*DK), T]
                wqt_sb = qh.tile([P, 4, Tq], BF16, tag="qh")
                wkt_sb = p1.tile([P, 4, Tq], BF16, tag="wkt")
                for pair in range(4):
                    for th in range(NTH):
                        tsl = slice(th * TH, (th + 1) * TH)
                        pq = psum.tile([P, TH], FP32, tag="mm")
                        for dc in range(ND):
                            nc.tensor.matmul(
                                pq[:],
                                lhsT=wq_sb[:, dc, pair * P:(pair + 1) * P],
                                rhs=yT_sb[:, dc, tsl],
                                start=(dc == 0), stop=(dc == ND - 1),
                            )
                        nc.vector.tensor_copy(out=wqt_sb[:, pair, tsl], in_=pq[:])
                        pk = psum.tile([P, TH], FP32, tag="mm")
                        for dc in range(ND):
                            nc.tensor.matmul(
                                pk[:],
                                lhsT=wk_sb[:, dc, pair * P:(pair + 1) * P],
                                rhs=kvT_sb[:, dc, tsl],
                                start=(dc == 0), stop=(dc == ND - 1),
                            )
                        nc.vector.tensor_copy(out=wkt_sb[:, pair, tsl], in_=pk[:])

                # ---- V projection, all 8 heads at once: WV[s, hv]
                wv_all = p1.tile([P, NT, cfg.HV], BF16, tag="wv_all")
                for st in range(NT):
                    pv = psum.tile([P, cfg.HV], FP32, tag="mm")
                    for dc in range(ND):
                        nc.tensor.matmul(
                            pv[:],
                            lhsT=kvT_sb[:, dc, st * P:(st + 1) * P],
                            rhs=wv_sb[:, dc, :],
                            start=(dc == 0), stop=(dc == ND - 1),
                        )
                    nc.vector.tensor_copy(out=wv_all[:, st, :], in_=pv[:])

                # ---- per head: scoresT -> exp (+denom) -> fold into WV -> partialT
                pt_sb = p1.tile([P, 4, Tq], BF16, tag="pt")
                for h in range(8):
                    pair, j = h // 2, h % 2
                    lo, hi = 64 * j, 64 * j + 64
                    exp_sb = expp.tile([P, NT, Tq], BF16, tag="exp")
                    denom = small.tile([P, NT], FP32, tag="denom")
                    for st in range(NT):
                        ps = psc.tile([P, Tq], FP32, tag="sc")
                        for th in range(NTH):
                            nc.tensor.matmul(
                                ps[:, th * TH:(th + 1) * TH],
                                lhsT=wkt_sb[lo:hi, pair, st * P:(st + 1) * P],
                                rhs=wqt_sb[lo:hi, pair, th * TH:(th + 1) * TH],
                                start=True, stop=True,
                            )
                        nc.scalar.activation(
                            out=exp_sb[:, st, :], in_=ps[:],
                            func=mybir.ActivationFunctionType.Exp,
                            accum_out=denom[:, st:st + 1],
                        )
                    rden = small.tile([P, NT], FP32, tag="rden")
                    nc.vector.reciprocal(out=rden[:], in_=denom[:])
                    wvp = small.tile([P, NT, DV], BF16, tag="wvp")
                    for st in range(NT):
                        nc.vector.tensor_scalar_mul(
                            out=wvp[:, st, :],
                            in0=wv_all[:, st, 64 * h:64 * h + 64],
                            scalar1=rden[:, st:st + 1],
                        )
                    for th in range(NTH):
                        tsl = slice(th * TH, (th + 1) * TH)
                        pp = psum.tile([P, TH], FP32, tag="mm")
                        for st in range(NT):
                            nc.tensor.matmul(
                                pp[:64, :],
                                lhsT=wvp[:, st, :],
                                rhs=exp_sb[:, st, tsl],
                                start=(st == 0), stop=(st == NT - 1),
                            )
                        nc.vector.tensor_copy(
                            out=pt_sb[lo:hi, pair, tsl], in_=pp[:64, :]
                        )

                # ---- Wo row-slice: partial [t, d] (+ y/2 if mha1)
                for tt in range(NT):
                    mo = rows.tile([P, DN], FP32, tag="rows")
                    yh_t = None
                    if add_yh:
                        yh_t = rows.tile([P, DN], FP32, tag="rows")
                        nc.sync.dma_start(yh_t[:], yh[:, tt, :])
                    for dh in range(NDH):
                        po = psum.tile([P, TH], FP32, tag="mm")
                        for i in range(4):
                            nc.tensor.matmul(
                                po[:],
                                lhsT=pt_sb[:, i, tt * P:(tt + 1) * P],
                                rhs=wo_sb[:, i, dh * TH:(dh + 1) * TH],
                                start=(i == 0), stop=(i == 3),
                            )
                        sl = slice(dh * TH, (dh + 1) * TH)
                        if add_yh:
                            nc.vector.tensor_add(out=mo[:, sl], in0=po[:], in1=yh_t[:, sl])
                        else:
                            nc.vector.tensor_copy(out=mo[:, sl], in_=po[:])
                    row0 = ((tt // HT) * 3 + branch) * TH + (tt % HT) * P
                    nc.sync.dma_start(bounce_in[row0:row0 + P, :], mo[:])

            mha(wq1, wk1, wv1, wo1, yT_sb, 0, True)
            mha(wq2, wk2, wv2, wo2, xT_sb, 1, False)

            # ---- FFN, one T-half at a time:
            #      h1T[f,t] = relu(W_in @ y^T + b_in);  ffp = h1^T @ W_out^T (+b_out/2)
            wot_sb = xw.tile([P, NF, DN], BF16, tag="xw")
            nc.sync.dma_start(wot_sb[:], wot[:])
            for th in range(NTH):
                tsl = slice(th * TH, (th + 1) * TH)
                h1_sb = qh.tile([P, NF, TH], BF16, tag="qh")
                for fc in range(NF):
                    wi_c = wic.tile([P, ND, P], BF16, tag="wic")
                    nc.sync.dma_start(wi_c[:], wi[:, :, fc * P:(fc + 1) * P])
                    ph = psum.tile([P, TH], FP32, tag="mm")
                    for dc in range(ND):
                        nc.tensor.matmul(
                            ph[:],
                            lhsT=wi_c[:, dc, :],
                            rhs=yT_sb[:, dc, tsl],
                            start=(dc == 0), stop=(dc == ND - 1),
                        )
                    nc.scalar.activation(
                        out=h1_sb[:, fc, :], in_=ph[:],
                        func=mybir.ActivationFunctionType.Relu,
                        bias=bi_sb[:, fc:fc + 1],
                    )
                for tl in range(HT):
                    tt = th * HT + tl
                    fo = rows.tile([P, DN], FP32, tag="rows")
                    for dh in range(NDH):
                        pf = psum.tile([P, TH], FP32, tag="mm")
                        for fc in range(NF):
                            nc.tensor.matmul(
                                pf[:],
                                lhsT=h1_sb[:, fc, tl * P:(tl + 1) * P],
                                rhs=wot_sb[:, fc, dh * TH:(dh + 1) * TH],
                                start=(fc == 0), stop=(fc == NF - 1),
                            )
                        sl = slice(dh * TH, (dh + 1) * TH)
                        nc.vector.tensor_add(out=fo[:, sl], in0=pf[:], in1=bo_sb[:, sl])
                    row0 = (th * 3 + 2) * TH + tl * P
                    nc.sync.dma_start(bounce_in[row0:row0 + P, :], fo[:])

            # ---- ReduceScatter over core pairs
            nc.gpsimd.collective_compute(
                "ReduceScatter",
                mybir.AluOpType.add,
                replica_groups=[[2 * g, 2 * g + 1] for g in range(n_cores // 2)],
                ins=[bounce_in.opt()],
                outs=[bounce_out.opt()],
            )

            # ---- tail: sub_norm chain on own t-half
            nsub = max(1, DN // 512)
            sub = DN // nsub

            def sub_norm(x_sb):
                stats = small.tile([P, nsub, 6], FP32, tag="stats")
                for i in range(nsub):
                    nc.vector.bn_stats(
                        out=stats[:, i, :], in_=x_sb[:, i * sub:(i + 1) * sub]
                    )
                mv = small.tile([P, 2], FP32, tag="mv")
                nc.vector.bn_aggr(out=mv[:], in_=stats[:])
                std = small.tile([P, 1], FP32, tag="std")
                nc.scalar.activation(
                    out=std[:], in_=mv[:, 1:2],
                    func=mybir.ActivationFunctionType.Sqrt,
                    scale=float(DN) / float(DN - 1),
                )
                msum = small.tile([P, 1], FP32, tag="msum")
                nc.vector.tensor_add(out=msum[:], in0=mv[:, 0:1], in1=std[:])
                nc.vector.tensor_scalar_sub(out=x_sb[:], in0=x_sb[:], scalar1=msum[:])

            for j in range(HT):
                m1t = rows.tile([P, DN], FP32, tag="rows")
                nc.sync.dma_start(m1t[:], bounce_out[0 * TH + j * P:0 * TH + j * P + P, :])
                m2t = rows.tile([P, DN], FP32, tag="rows")
                nc.sync.dma_start(m2t[:], bounce_out[1 * TH + j * P:1 * TH + j * P + P, :])
                mft = rows.tile([P, DN], FP32, tag="rows")
                nc.sync.dma_start(mft[:], bounce_out[2 * TH + j * P:2 * TH + j * P + P, :])
                sub_norm(m1t)                                  # out1
                nc.vector.tensor_add(out=m2t[:], in0=m2t[:], in1=m1t[:])
                sub_norm(m2t)                                  # out2
                nc.vector.tensor_add(out=mft[:], in0=mft[:], in1=m2t[:])
                sub_norm(mft)                                  # out
                nc.sync.dma_start(out[j * P:(j + 1) * P, :], mft[:])

    if compile:
        nc.compile()
    return nc


# ---------------------------------------------------------------- host side

def pack_inputs(cfg, x, y, Wq1, Wk1, Wv1, Wo1, Wq2, Wk2, Wv2, Wo2,
                W_in, b_in, W_out, b_out):
    """Build the per-core input maps.  Core half h owns heads [8h, 8h+8)."""
    Tq, DN, FFH, ND, NT, NF = cfg.T, cfg.D, cfg.FFH, cfg.ND, cfg.NT, cfg.NF
    NH = H // 2               # heads per core

    def tr_bf16(a):   # [T, D] -> [P, ND, T]:  out[p,dc,t] = a[t, dc*P+p]
        return np.ascontiguousarray(
            a.T.reshape(ND, P, Tq).transpose(1, 0, 2)
        ).astype(NPBF16)

    def qk_pack(W, h0):  # [H, D, DK] -> [P, ND, NH*DK]; pair-major free idx
        Wh = W[h0:h0 + NH]                          # [NH, D, DK]
        Wp = Wh.reshape(NH // 2, 2, DN, DK).transpose(2, 0, 1, 3)  # [D,pair,2,DK]
        Wp = Wp.reshape(DN, NH * DK)
        return np.ascontiguousarray(
            Wp.reshape(ND, P, NH * DK).transpose(1, 0, 2)
        ).astype(NPBF16)

    def v_pack(W, h0):  # [H, D, DV] -> [P, ND, NH*DV] with hv = h'*DV+v
        Wh = W[h0:h0 + NH].transpose(1, 0, 2).reshape(DN, NH * DV)
        return np.ascontiguousarray(
            Wh.reshape(ND, P, NH * DV).transpose(1, 0, 2)
        ).astype(NPBF16)

    def wo_pack(Wo, h):  # [D, D] -> [P, 4, D]: rows [NH*DV*h, +NH*DV)
        Ws = Wo[NH * DV * h:NH * DV * h + NH * DV]  # [512, D]
        return np.ascontiguousarray(
            Ws.reshape(4, P, DN).transpose(1, 0, 2)
        ).astype(NPBF16)

    def wi_pack(W_in, h):  # [FF, D] -> [P, ND, FFH]  lhsT[d, f]
        Ws = W_in[FFH * h:FFH * h + FFH]            # [FFH, D]
        return np.ascontiguousarray(
            Ws.T.reshape(ND, P, FFH).transpose(1, 0, 2)
        ).astype(NPBF16)

    def wot_pack(W_out, h):  # [D, FF] -> [P, NF, D]  moving[f, d]
        Ws = W_out[:, FFH * h:FFH * h + FFH].T      # [FFH, D]
        return np.ascontiguousarray(
            Ws.reshape(NF, P, DN).transpose(1, 0, 2)
        ).astype(NPBF16)

    scale = np.float32(1.0 / np.sqrt(np.float32(DK)))
    in_maps = []
    for c in range(2 * x.shape[0]):
        b, h = c // 2, c % 2
        h0 = NH * h
        m = dict(
            yT=tr_bf16(y[b]),
            xT=tr_bf16(x[b]),
            yh=np.ascontiguousarray(
                (0.5 * y[b]).reshape(NT, P, DN).transpose(1, 0, 2)
            ).astype(np.float32),
            wq1=qk_pack(Wq1 * scale, h0),
            wk1=qk_pack(Wk1, h0),
            wv1=v_pack(Wv1, h0),
            wo1=wo_pack(Wo1, h),
            wq2=qk_pack(Wq2 * scale, h0),
            wk2=qk_pack(Wk2, h0),
            wv2=v_pack(Wv2, h0),
            wo2=wo_pack(Wo2, h),
            wi=wi_pack(W_in, h),
            wot=wot_pack(W_out, h),
            bi=np.ascontiguousarray(
                b_in[FFH * h:FFH * h + FFH].reshape(NF, P).T
            ).astype(np.float32),
            bo=(0.5 * b_out).reshape(1, DN).astype(np.float32),
        )
        in_maps.append(m)
    return in_maps


_PROG_CACHE = {}


def kernel(**inputs) -> np.ndarray:
    cfg = Cfg()
    inputs = {k: np.asarray(v, np.float32) for k, v in inputs.items()}
    if "full" not in _PROG_CACHE:
        _PROG_CACHE["full"] = build_program(cfg)
    nc = _PROG_CACHE["full"]
    in_maps = pack_inputs(cfg, **inputs)
    res = run_bass_kernel_spmd(nc, in_maps, core_ids=list(range(8)))
    TH = cfg.T // 2
    out = np.empty((B, T, D), np.float32)
    for c in range(8):
        b, h = c // 2, c % 2
        out[b, h * TH:(h + 1) * TH] = res.results[c]["out"]
    return out
